# revision 18
# baseline (speedup 1.0000x reference)
"""Trainium2 Bass kernel for nn_CachedMoEExperts (MoE routing, E=16, top-4).

Strategy (expert-parallel, host-side dispatch):
  - Host computes the (tiny) router: softmax -> top-4 -> renormalize.
  - Tokens are gathered per expert on the host; experts are paired
    big-with-small and assigned 2 per NeuronCore (16 experts / 8 cores).
  - Each core runs the expert FFN y = gate * (w2 @ silu(w1 @ x_g^T)) for its
    two experts in fp16 (full-rate PE + fast weight load) on zero-padded
    token batches (slot capacities C0/C1, fixed at compile time).
  - Weights and gathered activations are host-packed into the exact SBUF
    tile layouts so every DMA is one fully-contiguous transfer.
  - Host scatter-adds the per-expert outputs back into the [T, H] result.

Engine map: PE matmuls; SP queue streams weights; gpsimd (SWDGE) queue loads
xg/gates so they prefetch past the in-order weight queue; DVE applies the
gate multiply (PSUM->SBUF); Act does silu + the y store.
"""

from contextlib import ExitStack

import numpy as np

import concourse.bacc as bacc
import concourse.bass as bass
import concourse.mybir as mybir
import concourse.tile as tile
from concourse.bass_utils import run_bass_kernel_spmd

F32 = mybir.dt.float32
FP16 = mybir.dt.float16

NUM_EXPERTS = 16
TOP_K = 4
HIDDEN = 2048
INTER = 1408
TOKENS = 4096
N_CORES = 8

KT1 = HIDDEN // 128  # 16 contraction tiles for mm1
MT1 = INTER // 128   # 11 output-row tiles for mm1
KT2 = INTER // 128   # 11 contraction tiles for mm2
MT2 = HIDDEN // 128  # 16 output-row tiles for mm2

# Default slot capacities (tokens routed per expert; avg load is T*K/E=1024).
CAP0_DEFAULT = 1120  # the 8 most-loaded experts
CAP1_DEFAULT = 1024  # the 8 least-loaded experts

_PROGRAM_CACHE: dict = {}


def _ceil32(n: int) -> int:
    return max(128, (int(n) + 31) // 32 * 32)


def _plan_chunks(C: int):
    """Split the token capacity into moving-dim chunks of <=512 (PSUM bank
    limit for fp32 accumulation)."""
    chunks = []
    off, rem = 0, C
    while rem > 0:
        sz = min(512, rem)
        chunks.append((off, sz))
        off += sz
        rem -= sz
    return chunks


def _build_program(C0: int, C1: int, reps: int = 1, xg_q: int = 4):
    caps = (C0, C1)
    nc = bacc.Bacc("TRN2", debug=False, target_bir_lowering=False)

    xg_d = [
        nc.dram_tensor(f"xg{s}", (128, KT1, caps[s]), FP16, kind="ExternalInput")
        for s in range(2)
    ]
    g_d = [
        nc.dram_tensor(f"g{s}", (128, caps[s]), F32, kind="ExternalInput")
        for s in range(2)
    ]
    y_d = [
        nc.dram_tensor(f"y{s}", (HIDDEN, caps[s]), FP16, kind="ExternalOutput")
        for s in range(2)
    ]
    w1_d = nc.dram_tensor(
        "w1p", (2, MT1, 128, KT1, 128), FP16, kind="ExternalInput"
    )
    w2_d = nc.dram_tensor(
        "w2p", (2, MT2, 128, KT2, 128), FP16, kind="ExternalInput"
    )

    with tile.TileContext(nc) as tc, ExitStack() as ctx:
        xgp = ctx.enter_context(tc.tile_pool(name="xg", bufs=1))
        wp = ctx.enter_context(tc.tile_pool(name="w", bufs=3))
        h1p = ctx.enter_context(tc.tile_pool(name="h1", bufs=1))
        gp = ctx.enter_context(tc.tile_pool(name="g", bufs=2))
        pp = ctx.enter_context(
            tc.tile_pool(name="psum", bufs=6, space=bass.MemorySpace.PSUM)
        )
        op = ctx.enter_context(tc.tile_pool(name="out", bufs=3))
        if reps > 1:
            ctx.enter_context(tc.For_i(0, reps, 1))

        for s in range(2):
            C = caps[s]
            chunks = _plan_chunks(C)

            xg_t = xgp.tile([128, KT1, C], FP16, tag=f"xg{s}", name=f"xg_s{s}")
            # sub-block loads so mm1 can start after the first kt block lands;
            # finer blocks up front let the PE start sooner
            if xg_q == 4:
                kq_blocks = [(0, 1), (1, 1), (2, 2), (4, 4), (8, 4), (12, 4)]
            else:
                kq_blocks = [(k, xg_q) for k in range(0, KT1, xg_q)]
            for kq, kn in kq_blocks:
                nc.gpsimd.dma_start(
                    xg_t[:, kq : kq + kn, :], xg_d[s].ap()[:, kq : kq + kn, :]
                )
            g_t = gp.tile([128, C], F32, tag="g", name=f"g_s{s}")
            nc.gpsimd.dma_start(g_t[:], g_d[s].ap()[:, :])

            h1_tiles = [
                h1p.tile([128, C], FP16, tag=f"h1_{m}", name=f"h1_s{s}_{m}")
                for m in range(MT1)
            ]

            # mm1 + silu: h1[i, t] = silu(sum_h w1[i, h] * x[t, h])
            # m-pairs with kt-major inner order: while xg streams in, the PE
            # has 2 m-tiles of ready work per delivered kt block, so it does
            # not starve during the initial delivery.
            for m0 in range(0, MT1, 2):
                ms = [m for m in (m0, m0 + 1) if m < MT1]
                wts, pss = {}, {}
                for mi, m in enumerate(ms):
                    wts[m] = wp.tile([128, KT1, 128], FP16, tag=f"w1_{m % 2}",
                                     bufs=3, name=f"w1_s{s}_{m}")
                    nc.sync.dma_start(wts[m][:], w1_d.ap()[s, m])
                    for ci, (off, szn) in enumerate(chunks):
                        pss[(m, ci)] = pp.tile(
                            [128, szn], F32, tag=f"ps{mi}{ci}", bufs=1,
                            name=f"ps1_s{s}_{m}_{ci}",
                        )
                for kt in range(KT1):
                    for m in ms:
                        for ci, (off, szn) in enumerate(chunks):
                            nc.tensor.matmul(
                                pss[(m, ci)][:],
                                wts[m][:, kt, :],
                                xg_t[:, kt, off : off + szn],
                                start=(kt == 0),
                                stop=(kt == KT1 - 1),
                            )
                for m in ms:
                    for ci, (off, szn) in enumerate(chunks):
                        nc.scalar.activation(
                            h1_tiles[m][:, off : off + szn],
                            pss[(m, ci)][:],
                            mybir.ActivationFunctionType.Silu,
                        )

            # mm2 + gate: y[hh, t] = g[t] * sum_i w2[hh, i] * h1[i, t]
            # reuses mm1's psum tags in alternating sets for double buffering
            for m2 in range(MT2):
                wt2 = wp.tile([128, KT2, 128], FP16, tag="w2", bufs=4,
                              name=f"w2_s{s}_{m2}")
                nc.sync.dma_start(wt2[:], w2_d.ap()[s, m2])
                pss = [
                    pp.tile([128, szn], F32, tag=f"ps{(m2 + 1) % 2}{ci}", bufs=1,
                            name=f"ps2_s{s}_{m2}_{ci}")
                    for ci, (off, szn) in enumerate(chunks)
                ]
                for kt in range(KT2):
                    for ci, (off, szn) in enumerate(chunks):
                        nc.tensor.matmul(
                            pss[ci][:],
                            wt2[:, kt, :],
                            h1_tiles[kt][:, off : off + szn],
                            start=(kt == 0),
                            stop=(kt == KT2 - 1),
                        )
                ot = op.tile([128, C], FP16, tag="out", name=f"ot_s{s}_{m2}")
                for ci, (off, szn) in enumerate(chunks):
                    nc.vector.tensor_mul(
                        ot[:, off : off + szn], pss[ci][:],
                        g_t[:, off : off + szn],
                    )
                nc.scalar.dma_start(
                    y_d[s].ap()[m2 * 128 : (m2 + 1) * 128, :], ot[:]
                )

    nc.compile()
    return nc


def _get_program(C0: int, C1: int):
    key = (C0, C1)
    if key not in _PROGRAM_CACHE:
        _PROGRAM_CACHE[key] = _build_program(C0, C1)
    return _PROGRAM_CACHE[key]


def _route(router_logits: np.ndarray):
    """softmax -> top-4 (desc, ties by lower index) -> renormalize; matches
    jax.nn.softmax + jax.lax.top_k semantics in float32."""
    logits = router_logits.astype(np.float32, copy=False)
    m = logits.max(axis=-1, keepdims=True)
    e = np.exp(logits - m)
    probs = e / e.sum(axis=-1, keepdims=True)
    top_idx = np.argsort(-probs, axis=-1, kind="stable")[:, :TOP_K]
    top_vals = np.take_along_axis(probs, top_idx, axis=-1)
    top_vals = top_vals / top_vals.sum(axis=-1, keepdims=True)
    return top_idx.astype(np.int64), top_vals.astype(np.float32)


def _pack_w1(w1e: np.ndarray) -> np.ndarray:
    # [I, H] -> [MT1, 128, KT1, 128] with [m, p, kt, j] = w1e[m*128+j, kt*128+p]
    return np.ascontiguousarray(
        w1e.reshape(MT1, 128, KT1, 128).transpose(0, 3, 2, 1).astype(np.float16)
    )


def _pack_w2(w2e: np.ndarray) -> np.ndarray:
    # [H, I] -> [MT2, 128, KT2, 128] with [m, p, kt, j] = w2e[m*128+j, kt*128+p]
    return np.ascontiguousarray(
        w2e.reshape(MT2, 128, KT2, 128).transpose(0, 3, 2, 1).astype(np.float16)
    )


def _pack_xg(xsel: np.ndarray, C: int) -> np.ndarray:
    # xsel [n, H] -> [128, KT1, C] with [p, kt, t] = xsel[t, kt*128+p]
    n = xsel.shape[0]
    out = np.zeros((128, KT1, C), np.float16)
    out[:, :, :n] = xsel.T.reshape(KT1, 128, n).transpose(1, 0, 2)
    return out


def _prepare(x, router_logits, w1, w2):
    x = np.ascontiguousarray(np.asarray(x, dtype=np.float32))
    router_logits = np.asarray(router_logits, dtype=np.float32)
    w1 = np.asarray(w1, dtype=np.float32)
    w2 = np.asarray(w2, dtype=np.float32)
    T = x.shape[0]

    top_idx, top_gates = _route(router_logits)

    flat_e = top_idx.ravel()
    flat_t = np.repeat(np.arange(T), TOP_K)
    flat_g = top_gates.ravel()
    order = np.argsort(flat_e, kind="stable")
    st, sg = flat_t[order], flat_g[order]
    counts = np.bincount(flat_e, minlength=NUM_EXPERTS)
    starts = np.concatenate([[0], np.cumsum(counts)])
    toks = [st[starts[e] : starts[e + 1]] for e in range(NUM_EXPERTS)]
    gs = [sg[starts[e] : starts[e + 1]] for e in range(NUM_EXPERTS)]

    # pair the most-loaded expert with the least-loaded, 2 experts per core
    rank = np.argsort(-counts, kind="stable")
    big = rank[:N_CORES]
    small = rank[N_CORES:][::-1]  # big[i] pairs with small[i]

    C0 = max(CAP0_DEFAULT, _ceil32(counts[big].max()))
    C1 = max(CAP1_DEFAULT, _ceil32(counts[small].max()))
    nc = _get_program(C0, C1)

    in_maps = []
    for c in range(N_CORES):
        pair = (int(big[c]), int(small[c]))
        caps = (C0, C1)
        im = {}
        for s, e in enumerate(pair):
            n = int(counts[e])
            im[f"xg{s}"] = _pack_xg(x[toks[e]], caps[s])
            g = np.zeros((caps[s],), np.float32)
            g[:n] = gs[e]
            im[f"g{s}"] = np.broadcast_to(g, (128, caps[s])).copy()
        im["w1p"] = np.stack([_pack_w1(w1[e]) for e in pair])
        im["w2p"] = np.stack([_pack_w2(w2[e]) for e in pair])
        in_maps.append(im)

    meta = dict(T=T, counts=counts, toks=toks, big=big, small=small)
    return nc, in_maps, meta


def _combine(results, meta):
    out = np.zeros((meta["T"], HIDDEN), np.float32)
    for c in range(N_CORES):
        for s, e in enumerate((int(meta["big"][c]), int(meta["small"][c]))):
            n = int(meta["counts"][e])
            y = results[c][f"y{s}"]  # [HIDDEN, Cs], already gate-scaled
            out[meta["toks"][e]] += y[:, :n].T.astype(np.float32)
    return out


def kernel(x, router_logits, w1, w2):
    nc, in_maps, meta = _prepare(x, router_logits, w1, w2)
    res = run_bass_kernel_spmd(nc, in_maps, core_ids=list(range(N_CORES)))
    kernel._last_results = res
    return _combine(res.results, meta)
